# revision 1
# baseline (speedup 1.0000x reference)
"""Trainium2 Bass kernel for the CNN-VAE loss:

    prob = einsum('klb,hwb->klhw', beta, A) * 5000
    mse  = mean(sum(|x - prob[:, :, None]|^2, axis=1))

Strategy
--------
K*L = 128 == SBUF partition count, so (k,l) lives on partitions and the
40000-pixel hw axis is sharded across the 8 cores (5000 pixels each);
every core sees all 128 (k,l) rows and all 3 channels of its hw slice.

Per core, pipelined over 5 groups of 1000 pixels:
  PE:   prob group = (5000*beta)^T .T @ A^T, two 500-col fp32 matmuls
        into the two banks of a (128,1000) PSUM tile
        (lhsT = scaled beta^T (3,128) stationary, rhs = A^T (3,500))
  DVE:  x -= prob  (one in-place (128,3,1000) subtract per group; the
        PSUM prob tile is broadcast over the channel dim via a step-0 AP)
  ACT:  x = Square(x) in place, accum_out -> per-partition sum column
The (128,5) accumulator is DMA'd out; the host sums partials across
columns, partitions, and cores, and divides by 16*3*200*200 (the mean
denominator; the sum over L is folded into the partition-dim sum).

A^T and beta^T are concatenated into a single (3, 5128) constant input
so one early DMA delivers both before the 7.7MB x stream saturates the
DMA engines.
"""

import numpy as np

K, L, NB, H, W = 16, 8, 3, 200, 200
KL = K * L          # 128 partitions
C = 3               # broadcast channel dim of x
HW = H * W          # 40000
N_CORES = 8
HW_SHARD = HW // N_CORES   # 5000
MCHUNK = 500               # matmul chunk (one PSUM bank)
GROUP = 1000               # pixels per steady-state iteration
N_GROUPS = HW_SHARD // GROUP    # 5
CONST_W = HW_SHARD + KL    # 5128: A^T shard columns + beta^T columns
SCALE = 5000.0
DENOM = float(K * C * H * W)  # mean over [K, C, H, W] after summing L

_NC = None


def _build():
    global _NC
    if _NC is not None:
        return _NC
    from contextlib import ExitStack

    import concourse.bacc as bacc
    import concourse.mybir as mybir
    import concourse.tile as tile

    f32 = mybir.dt.float32
    nc = bacc.Bacc("TRN2", target_bir_lowering=False, debug=False)

    xs = nc.dram_tensor("xs", [KL, C, HW_SHARD], f32, kind="ExternalInput").ap()
    cb = nc.dram_tensor("cb", [NB, CONST_W], f32, kind="ExternalInput").ap()
    out = nc.dram_tensor("out", [KL, N_GROUPS], f32, kind="ExternalOutput").ap()

    with tile.TileContext(nc) as tc, ExitStack() as ctx:
        const = ctx.enter_context(tc.tile_pool(name="const", bufs=1))
        xpool = ctx.enter_context(tc.tile_pool(name="x", bufs=4))
        ppool = ctx.enter_context(tc.tile_pool(name="psum", bufs=4, space="PSUM"))

        cb_sb = const.tile([NB, CONST_W], f32)
        nc.sync.dma_start(cb_sb[:], cb[:])
        bts = const.tile([NB, KL], f32)
        nc.vector.tensor_scalar_mul(bts[:], cb_sb[:, HW_SHARD:CONST_W], SCALE)

        acc = const.tile([KL, N_GROUPS], f32)

        BANK = 512  # PSUM bank width in f32; matmul output must stay in-bank
        for g in range(N_GROUPS):
            pp = ppool.tile([KL, 2 * BANK], f32)  # two PSUM banks
            for h in range(GROUP // MCHUNK):
                ci = g * (GROUP // MCHUNK) + h
                nc.tensor.matmul(
                    pp[:, h * BANK : h * BANK + MCHUNK],
                    bts[:],
                    cb_sb[:, ci * MCHUNK : (ci + 1) * MCHUNK],
                    start=True,
                    stop=True,
                )
            xt = xpool.tile([KL, C, GROUP], f32)
            nc.sync.dma_start(xt[:], xs[:, :, g * GROUP : (g + 1) * GROUP])
            pv = pp[:].rearrange("p (u f) -> p u f", f=BANK)[:, :, :MCHUNK]
            prob_b = pv.unsqueeze(1).broadcast_to([KL, C, 2, MCHUNK])
            xv = xt[:].rearrange("p c (u f) -> p c u f", f=MCHUNK)
            nc.vector.tensor_sub(xv, xv, prob_b)
            nc.scalar.activation(
                xt[:],
                xt[:],
                mybir.ActivationFunctionType.Square,
                accum_out=acc[:, g : g + 1],
            )

        nc.sync.dma_start(out[:], acc[:])

    nc.compile()
    _NC = nc
    return nc


def _make_in_maps(x, beta, A):
    x = np.ascontiguousarray(np.asarray(x, dtype=np.float32))
    beta = np.ascontiguousarray(np.asarray(beta, dtype=np.float32))
    A = np.ascontiguousarray(np.asarray(A, dtype=np.float32))

    xr = x.reshape(KL, C, HW)
    at_full = A.reshape(HW, NB).T          # (3, 40000)
    bt = beta.reshape(KL, NB).T            # (3, 128)

    in_maps = []
    for i in range(N_CORES):
        sl = slice(i * HW_SHARD, (i + 1) * HW_SHARD)
        cb = np.concatenate([at_full[:, sl], bt], axis=1)  # (3, 5128)
        in_maps.append(
            {
                "xs": np.ascontiguousarray(xr[:, :, sl]),
                "cb": np.ascontiguousarray(cb),
            }
        )
    return in_maps


def _run(in_maps, trace=False, **kwargs):
    from concourse import bass_utils

    nc = _build()
    return bass_utils.run_bass_kernel_spmd(
        nc, in_maps, list(range(N_CORES)), trace=trace, **kwargs
    )


def _combine(results):
    total = 0.0
    for r in results:
        total += float(np.sum(np.asarray(r["out"], dtype=np.float64)))
    return np.float32(total / DENOM)


def kernel(x, beta, A):
    res = _run(_make_in_maps(x, beta, A))
    return _combine(res.results)



# revision 5
# speedup vs baseline: 1.2708x; 1.2708x over previous
"""Trainium2 Bass kernel for the CNN-VAE loss:

    prob = einsum('klb,hwb->klhw', beta, A) * 5000
    mse  = mean(sum(|x - prob[:, :, None]|^2, axis=1))

Strategy (v2: algebraic expansion, bf16 stream)
-----------------------------------------------
Expand  sum |x - p|^2 = sum x^2 - 2*sum x*p + C*sum p^2  (p broadcast over
the C=3 channel dim).  With p = SCALE * einsum('klb,hwb', beta, A):

  T1 = sum x^2                                   -> device (per-partition
       accumulators; split between ACT Square+accum and DVE fused
       multiply-reduce so neither engine exceeds the DMA window)
  T2 = -2*SCALE * sum_b sum_pix A[pix,b]*Y[b,pix],
       Y[b,pix] = sum_{kl,c} beta[kl,b] * x[kl,c,pix]
       -> Y computed on the otherwise-idle PE: beta (128x3 bf16) is the
       stationary operand, x streams through as the moving operand, and
       the c-sum is folded into the PSUM accumulation (3 matmuls/chunk).
       The A-weighted pixel reduction is one fused DVE
       tensor_tensor_reduce straight out of PSUM.
  T3 = C*SCALE^2 * sum_kl beta^T (A^T A) beta   -> host, f64, from the
       tiny beta/A inputs (A^T A is 3x3 over 40k pixels).

x is uploaded as bf16 (halves HBM traffic; contributes <1e-9 relative
error vs the 2e-2 tolerance since T1+T2 are ~1e-7 of the total).  The
hw axis is sharded across the 8 cores (5000 pixels each); every core
sees all 128 (k,l) rows.  beta/A constants go out on the scalar-engine
HWDGE queue so they land before the x stream monopolizes the SDMA
engines.
"""

import numpy as np

K, L, NB, H, W = 16, 8, 3, 200, 200
KL = K * L          # 128 partitions
C = 3               # broadcast channel dim of x
HW = H * W          # 40000
N_CORES = 8
HW_SHARD = HW // N_CORES   # 5000
GROUP = 1000               # pixels per steady-state iteration
NG = HW_SHARD // GROUP     # 5
HALF = GROUP // 2          # 500: matmul free-dim chunk (fits one PSUM bank)
BANK = 512                 # PSUM bank width in f32
XCOLS = C * GROUP          # 3000 x columns per group
DVE_SQ = 1152              # x^2 columns handled by DVE (rest on ACT)
SCALE = 5000.0
DENOM = float(K * C * H * W)  # mean denominator (sum over L folded in)

_NC = None


def _build():
    global _NC
    if _NC is not None:
        return _NC
    from contextlib import ExitStack

    import concourse.bacc as bacc
    import concourse.mybir as mybir
    import concourse.tile as tile

    f32 = mybir.dt.float32
    bf16 = mybir.dt.bfloat16
    nc = bacc.Bacc("TRN2", target_bir_lowering=False, debug=False)

    xg = nc.dram_tensor("xg", [NG, KL, C, GROUP], bf16, kind="ExternalInput").ap()
    bsb = nc.dram_tensor("bsb", [KL, NB], bf16, kind="ExternalInput").ap()
    asb = nc.dram_tensor("asb", [NB, HW_SHARD], bf16, kind="ExternalInput").ap()
    out_sq = nc.dram_tensor("out_sq", [KL, 2 * NG], f32, kind="ExternalOutput").ap()
    out_t2 = nc.dram_tensor("out_t2", [NB, 2 * NG], f32, kind="ExternalOutput").ap()

    with tile.TileContext(nc) as tc, ExitStack() as ctx:
        const = ctx.enter_context(tc.tile_pool(name="const", bufs=1))
        xpool = ctx.enter_context(tc.tile_pool(name="x", bufs=3))
        spool = ctx.enter_context(tc.tile_pool(name="scr", bufs=2))
        ppool = ctx.enter_context(tc.tile_pool(name="psum", bufs=4, space="PSUM"))

        # constants on the ACT HWDGE queue: issue ahead of the x stream
        b_sb = const.tile([KL, NB], bf16)
        nc.scalar.dma_start(b_sb[:], bsb[:])
        a_sb = const.tile([NB, HW_SHARD], bf16)
        nc.scalar.dma_start(a_sb[:], asb[:])

        acc_sq = const.tile([KL, 2 * NG], f32)
        acc_t2 = const.tile([NB, 2 * NG], f32)

        # warm the ACT Square spline table while DMAs are in flight
        warm = const.tile([KL, 8], f32)
        nc.vector.memset(warm[:], 0.0)
        nc.scalar.activation(warm[:], warm[:], mybir.ActivationFunctionType.Square)

        for g in range(NG):
            xt = xpool.tile([KL, C, GROUP], bf16)
            nc.sync.dma_start(xt[:], xg[g])

            # PE: Y[b, pix] += sum_kl beta[kl,b] * x[kl,c,pix], c folded
            # into the PSUM accumulation group
            yt = ppool.tile([NB, 2 * BANK], f32)
            for h in range(2):
                for c in range(C):
                    nc.tensor.matmul(
                        yt[:, h * BANK : h * BANK + HALF],
                        b_sb[:],
                        xt[:, c, h * HALF : (h + 1) * HALF],
                        start=(c == 0),
                        stop=(c == C - 1),
                    )

            # T2 partial: acc_t2[:, 2g+h] = sum_pix Y * A^T (fused mult+accum,
            # scalar_tensor_tensor reads Y straight out of PSUM)
            t2s = spool.tile([NB, GROUP], bf16)
            for h in range(2):
                nc.vector.scalar_tensor_tensor(
                    out=t2s[:, h * HALF : (h + 1) * HALF],
                    in0=yt[:, h * BANK : h * BANK + HALF],
                    scalar=1.0,
                    in1=a_sb[:, g * GROUP + h * HALF : g * GROUP + (h + 1) * HALF],
                    op0=mybir.AluOpType.mult,
                    op1=mybir.AluOpType.mult,
                    accum_out=acc_t2[:, 2 * g + h : 2 * g + h + 1],
                )

            # T1 partials: x^2 split between DVE (fused) and ACT (Square)
            xf = xt[:].rearrange("p c f -> p (c f)")
            sqs = spool.tile([KL, DVE_SQ], bf16)
            nc.vector.scalar_tensor_tensor(
                out=sqs[:],
                in0=xf[:, :DVE_SQ],
                scalar=1.0,
                in1=xf[:, :DVE_SQ],
                op0=mybir.AluOpType.mult,
                op1=mybir.AluOpType.mult,
                accum_out=acc_sq[:, NG + g : NG + g + 1],
            )
            sqa = spool.tile([KL, XCOLS - DVE_SQ], bf16)
            nc.scalar.activation(
                sqa[:],
                xf[:, DVE_SQ:],
                mybir.ActivationFunctionType.Square,
                accum_out=acc_sq[:, g : g + 1],
            )

        nc.sync.dma_start(out_sq[:], acc_sq[:])
        nc.sync.dma_start(out_t2[:], acc_t2[:])

    nc.compile()
    _NC = nc
    return nc


def _make_in_maps(x, beta, A):
    import ml_dtypes

    bf16 = ml_dtypes.bfloat16
    x = np.asarray(x, dtype=np.float32)
    beta = np.asarray(beta, dtype=np.float32)
    A = np.asarray(A, dtype=np.float32)

    # (KL, C, cores, NG, GROUP) -> (cores, NG, KL, C, GROUP)
    xr = x.reshape(KL, C, N_CORES, NG, GROUP).transpose(2, 3, 0, 1, 4)
    xb = np.ascontiguousarray(xr.astype(bf16))
    bt = np.ascontiguousarray(beta.reshape(KL, NB).astype(bf16))
    # A^T shards: (cores, NB, HW_SHARD)
    at = np.ascontiguousarray(
        A.reshape(N_CORES, HW_SHARD, NB).transpose(0, 2, 1).astype(bf16)
    )

    in_maps = []
    for i in range(N_CORES):
        in_maps.append(
            {
                "xg": np.ascontiguousarray(xb[i]),
                "bsb": bt,
                "asb": at[i],
            }
        )
    return in_maps


def _run(in_maps, trace=False, **kwargs):
    from concourse import bass_utils

    nc = _build()
    return bass_utils.run_bass_kernel_spmd(
        nc, in_maps, list(range(N_CORES)), trace=trace, **kwargs
    )


def _combine(results, beta, A):
    t1 = 0.0
    t2 = 0.0
    for r in results:
        t1 += float(np.sum(np.asarray(r["out_sq"], dtype=np.float64)))
        t2 += float(np.sum(np.asarray(r["out_t2"], dtype=np.float64)))
    bf = np.asarray(beta, dtype=np.float64).reshape(KL, NB)
    af = np.asarray(A, dtype=np.float64).reshape(HW, NB)
    m = af.T @ af  # 3x3
    t3 = float(C) * SCALE * SCALE * float(np.einsum("kb,bc,kc->", bf, m, bf))
    total = t1 - 2.0 * SCALE * t2 + t3
    return np.float32(total / DENOM)


def kernel(x, beta, A):
    res = _run(_make_in_maps(x, beta, A))
    return _combine(res.results, beta, A)
